# revision 25
# baseline (speedup 1.0000x reference)
"""Trainium2 Bass kernel for nn_CrossAttention (B=2, C=512, N=M=2048, H=8).

Sharding: batch*heads = 16 (b,h) pairs across 8 cores, 2 heads per core.
Cores 0-3 handle batch 0 (heads 0..7 in pairs), cores 4-7 batch 1.

The kernel is ScalarE-exp-bound (softmax needs 65536 exp rows/core at
0.833ns — a ~55us engine floor no other engine can take), so PE work is
restructured to fit under it:
  qT[d,n] = (Wq*SCALE).T @ x_b   (bf16)                           8192c
  kT[d,m] = Wk.T @ y_b           (bf16, f32r in SBUF)             8192c
  v2[m,d] = y_b.T-slices @ (Wv*(1+lw))  direct [m,d] layout,      8192c
            bf16, no PE transposes; ones cols give the denominator
  S^T[m,n] = kT.T-slices @ qT   (K=64 pairs tile_position-packed) 65536c
  P = exp(S^T) -> bf16          (ScalarE, 64x [128,1024] blocks)
  att[n, d|den] += P_slice.T @ v2[m]   n-major: 128 out partitions,
            65-row bf16 matmuls (half the m-major cost)           33280c
  att_nrm[n,d2] = att * recip(den)     (DVE, per-partition scalar)
  attT[d2,n] = transpose(att_nrm)      (PE, bf16 identity)         2048c
  outT_partial[c,n] = Wp_rows.T @ attT  (bf16) -> f16 partials     8192c

The depthwise conv (ksize=1) folds into Wv scaling + a host-side output
bias (bias' = bp + lb @ Wp, exact because softmax rows sum to 1).
Host sums the 4 per-batch f16 partials in f32 and adds bias'.

PSUM: psA 3x[128,1024] ring (scores/exp; also proj, v2, transposes and
outproj transients) = 6 banks; psB 2x[128,512] = 2 banks holding the
8 per-chunk attnout accumulators (4x65 cols per bank; only the first
matmul into a bank uses start=True — the bank-wide pending-zero then
zero-initializes each co-located accumulation group on first touch).

Chunk q's drain (normalize/transpose/outproj/DMA) is woven into chunk
q+1's first m-steps so the PE's in-order queue and the psA ring never
stall the score stream that feeds ScalarE.
"""

import os
import sys
import numpy as np
from contextlib import ExitStack

for _p in ("/root/.axon_site", "/root/.axon_site/_ro/trn_rl_repo",
           "/root/.axon_site/_ro/pypackages", "/opt/trn_rl_repo"):
    if os.path.isdir(_p) and _p not in sys.path:
        sys.path.append(_p)

B, C, N, M, H = 2, 512, 2048, 2048, 8
HD = C // H
SCALE = HD ** -0.5
NCORES = 8

_NC = None
LAST_RUN = None


def _build_program():
    from concourse import bacc
    import concourse.tile as tile
    import concourse.mybir as mybir
    from concourse.masks import make_identity

    F32 = mybir.dt.float32
    F32R = mybir.dt.float32r
    BF16 = mybir.dt.bfloat16
    F16 = mybir.dt.float16
    EXP = mybir.ActivationFunctionType.Exp
    MULT = mybir.AluOpType.mult

    nc = bacc.Bacc("TRN2", target_bir_lowering=False, debug=False,
                   num_devices=NCORES)

    xr = nc.dram_tensor("xr", [C, N], BF16, kind="ExternalInput").ap()
    yr = nc.dram_tensor("yr", [C, M], BF16, kind="ExternalInput").ap()
    wq_d = nc.dram_tensor("wq", [128, 512], BF16, kind="ExternalInput").ap()
    wk_d = nc.dram_tensor("wk", [128, 512], BF16, kind="ExternalInput").ap()
    wv_d = nc.dram_tensor("wv", [128, 512], BF16, kind="ExternalInput").ap()
    wp_d = nc.dram_tensor("wp", [128, C], BF16, kind="ExternalInput").ap()
    outT = nc.dram_tensor("outT", [C, N], F16, kind="ExternalOutput").ap()

    with tile.TileContext(nc) as tc, ExitStack() as ctx:
        sb = ctx.enter_context(tc.tile_pool(name="sb", bufs=1))
        ppool = ctx.enter_context(tc.tile_pool(name="ppool", bufs=20))
        npool = ctx.enter_context(tc.tile_pool(name="npool", bufs=4))
        apool = ctx.enter_context(tc.tile_pool(name="apool", bufs=2))
        spool = ctx.enter_context(tc.tile_pool(name="spool", bufs=2))
        opool = ctx.enter_context(tc.tile_pool(name="opool", bufs=2))
        psA = ctx.enter_context(tc.tile_pool(name="psA", bufs=3, space="PSUM"))
        psB = ctx.enter_context(tc.tile_pool(name="psB", bufs=2, space="PSUM"))

        # ---- constants (identity first: PE warmup gates on it) ----
        ident = sb.tile([128, 128], BF16, tag="ident")
        make_identity(nc, ident)
        # v2 group tiles: [m 128, mi 4, 130] bf16; cols 64/129 stay 1.0
        # (the softmax-denominator ones columns)
        v2g = [sb.tile([128, 4, 130], BF16, tag=f"v2g_{g}", name=f"v2g_{g}")
               for g in range(4)]

        def v2s(m, lo, hi):
            return v2g[m // 4][:, m % 4, lo:hi]
        # warm the exp table while DMAs stream
        warm = sb.tile([1, 32], F32, tag="warm")
        nc.scalar.activation(warm, ident[0:1, 0:32], EXP)
        # warm the PE clock so early projections run fast
        psw = psB.tile([128, 512], F32, tag="acc", name="psw")
        for _ in range(16):
            nc.tensor.matmul(psw[:, 0:128], ident, ident, start=True, stop=True)
        warm2 = sb.tile([128, 128], F32, tag="warm2")
        nc.vector.tensor_copy(warm2, psw[:, 0:128])
        for g in range(4):
            nc.gpsimd.memset(v2g[g], 1.0)

        wq_sb = sb.tile([128, 4, 128], BF16, tag="wq_sb")
        wk_sb = sb.tile([128, 4, 128], BF16, tag="wk_sb")
        wv_sb = sb.tile([128, 4, 128], BF16, tag="wv_sb")
        wp_sb = sb.tile([128, C], BF16, tag="wp_sb")

        y_sb = sb.tile([128, 4, M], BF16, tag="y_sb")
        x_sb = sb.tile([128, 4, N], BF16, tag="x_sb")

        def load_j(dst, src, j, kcs=None):
            js = slice(j * 512, (j + 1) * 512)
            ks = slice(0, 4) if kcs is None else slice(kcs[0], kcs[1])
            cs = slice(ks.start * 128, ks.stop * 128)
            nc.sync.dma_start(
                out=dst[:, ks, js],
                in_=src[cs, js].rearrange("(kc p) m -> p kc m", p=128))

        # DMA order = consumption order; one DMA per j-chunk.  The first
        # exp is gated by (wq,x_j0) -> qT and (wk,y_j0) -> kT; the q side
        # goes first so its ScalarE copy overlaps the k projection.
        nc.sync.dma_start(
            out=wq_sb, in_=wq_d.rearrange("p (kc m) -> p kc m", m=128))
        load_j(x_sb, xr, 0)
        nc.sync.dma_start(
            out=wk_sb, in_=wk_d.rearrange("p (kc m) -> p kc m", m=128))
        load_j(y_sb, yr, 0, kcs=(0, 3))
        load_j(y_sb, yr, 0, kcs=(3, 4))
        nc.sync.dma_start(
            out=wv_sb, in_=wv_d.rearrange("p (kc m) -> p kc m", m=128))
        load_j(y_sb, yr, 1)
        load_j(y_sb, yr, 2)
        load_j(y_sb, yr, 3)
        nc.sync.dma_start(out=wp_sb, in_=wp_d)
        load_j(x_sb, xr, 1)
        load_j(x_sb, xr, 2)
        load_j(x_sb, xr, 3)

        kT = sb.tile([128, M], F32R, tag="kT")
        qT = sb.tile([128, N], F32R, tag="qT")

        projst = {}

        def proj_half(dst, w_sb, src, j, name, half, use_act=False):
            if half == 0:
                projst[name] = psA.tile([128, 512], F32, tag="blk", name=name)
            ps = projst[name]
            for kc in (0, 1) if half == 0 else (2, 3):
                nc.tensor.matmul(ps, w_sb[:, kc, :],
                                 src[:, kc, j * 512:(j + 1) * 512],
                                 start=(kc == 0), stop=(kc == 3))
            if half == 1:
                if use_act:
                    nc.scalar.copy(dst[:, j * 512:(j + 1) * 512], ps)
                else:
                    nc.vector.tensor_copy(dst[:, j * 512:(j + 1) * 512], ps)

        def proj(dst, w_sb, src, j, name, use_act=False):
            proj_half(dst, w_sb, src, j, name, 0, use_act)
            proj_half(dst, w_sb, src, j, name, 1, use_act)

        v2st = {}

        def v2_proj2(g, half):
            # four m-blocks share one PSUM bank; only the very first matmul
            # uses start=True (bank-wide pending-zero inits the rest)
            if half == 0:
                v2st[g] = psA.tile([128, 4, 128], F32, tag="blk",
                                   name=f"psv{g}")
            ps = v2st[g]
            for mi in (0, 1) if half == 0 else (2, 3):
                m = g * 4 + mi
                for kc in range(4):
                    nc.tensor.matmul(ps[:, mi, :],
                                     y_sb[:, kc, m * 128:(m + 1) * 128],
                                     wv_sb[:, kc, :],
                                     start=(mi == 0 and kc == 0),
                                     stop=(mi == 3 and kc == 3),
                                     skip_group_check=True)
            if half == 1:
                nc.vector.tensor_copy(v2g[g][:, :, 0:64], ps[:, :, 0:64])
                nc.vector.tensor_copy(v2g[g][:, :, 65:129], ps[:, :, 64:128])

        # ---- prologue: only the j0 projections gate the first exp;
        # qT copies on ScalarE while kT's goes to DVE in parallel; the
        # kT copy is split so the m0 scores wait only on cols 0:128 ----
        proj_half(qT, wq_sb, x_sb, 0, "psq0", 0, use_act=True)
        proj_half(qT, wq_sb, x_sb, 0, "psq0", 1, use_act=True)
        proj_half(kT, wk_sb, y_sb, 0, "psk0", 0)
        for kc in (2, 3):
            nc.tensor.matmul(projst["psk0"], wk_sb[:, kc, :],
                             y_sb[:, kc, 0:512],
                             start=(kc == 0), stop=(kc == 3))
        nc.vector.tensor_copy(kT[:, 0:128], projst["psk0"][:, 0:128])
        nc.vector.tensor_copy(kT[:, 128:512], projst["psk0"][:, 128:512])

        # fill task groups woven between score blocks (chunk -> per-m lists),
        # in units of <=~450ns of PE so the score stream never starves;
        # k-proj for chunk j must be fully emitted before scores m=4j read it
        def KJ(j, half):
            return lambda: proj_half(kT, wk_sb, y_sb, j, f"psk{j}", half)

        def QJ(j, half):
            return lambda: proj_half(qT, wq_sb, x_sb, j, f"psq{j}", half)

        def VG(g, half):
            return lambda: v2_proj2(g, half)

        fills = {
            0: [[VG(0, 0), VG(0, 1)],
                [KJ(1, 0), KJ(1, 1)],
                [],
                [VG(1, 0), VG(1, 1)],
                [],
                [KJ(2, 0), KJ(2, 1)],
                [],
                [VG(2, 0), VG(2, 1)],
                [],
                [KJ(3, 0), KJ(3, 1)],
                [],
                [VG(3, 0), VG(3, 1)],
                [QJ(1, 0), QJ(1, 1)]],
            1: [[QJ(2, 0), QJ(2, 1)]],
            2: [[QJ(3, 0), QJ(3, 1)]],
            3: [],
        }

        # ---- attention main loop ----
        from collections import deque
        aq = deque()         # (m, P, accA, accB) awaiting attnout
        drain = None         # [stage, chunk, state...] of the pending drain

        def emit_attnout(pm, pP, paccA, paccB):
            # pm==0/nb==0 is the first matmul into each fresh acc bank: its
            # start=True marks the whole bank pending-zero; later groups'
            # first writes then zero-init via the per-byte pending path.
            for h, acc in ((0, paccA), (1, paccB)):
                for nb in range(4):
                    nc.tensor.matmul(
                        acc[:, nb * 65:(nb + 1) * 65],
                        pP[:, h * 512 + nb * 128: h * 512 + (nb + 1) * 128],
                        v2s(pm, h * 65, h * 65 + 65),
                        start=(pm == 0 and nb == 0),
                        stop=(pm == 15 and nb == 3),
                        skip_group_check=True)

        def emit_norm(q, qaccA, qaccB):
            # batched strided reciprocal of the 4 denominator columns per
            # bank, then ONE broadcast tensor_tensor per bank: the [128,4]
            # reciprocals broadcast (stride-0) along the 64 d-columns
            nrm = npool.tile([128, 4, 128], BF16, tag="nrm", name=f"nrm{q}")
            for h, acc in ((0, qaccA), (1, qaccB)):
                rd = spool.tile([128, 4], F32, tag=f"rd{h}", name=f"rd{q}_{h}")
                nc.vector.reciprocal(rd, acc[:, 64:261:65])
                av = acc[:, 0:260].rearrange("p (nb c) -> p nb c", c=65)
                nc.vector.tensor_tensor(
                    nrm[:, :, h * 64:(h + 1) * 64], av[:, :, 0:64],
                    rd.to_broadcast([128, 4, 64]), op=MULT)
            return nrm

        def emit_transposes(q, nrm):
            # 4 transposes share one PSUM slot; one bf16 2x copy out
            tp = psA.tile([128, 512], BF16, tag="blk", name=f"tp{q}")
            for nb in range(4):
                nc.tensor.transpose(
                    tp[:, nb * 128:(nb + 1) * 128], nrm[:, nb, :], ident)
            at = apool.tile([128, 512], BF16, tag="attT", name=f"attT{q}")
            nc.vector.tensor_copy(at, tp)
            return at

        def emit_outproj(q, at, half, so, use_act=False):
            # two output-channel blocks share one PSUM slot -> f16 halves
            po = psA.tile([128, 1024], F32, tag="blk", name=f"po{q}_{half}")
            for i in range(2):
                cb = half * 2 + i
                for nb in range(4):
                    nc.tensor.matmul(
                        po[:, i * 512 + nb * 128: i * 512 + (nb + 1) * 128],
                        wp_sb[:, cb * 128:(cb + 1) * 128],
                        at[:, nb * 128:(nb + 1) * 128],
                        start=(nb == 0), stop=(nb == 3 and i == 1),
                        skip_group_check=True)
            if use_act:
                nc.scalar.copy(so[:, half * 1024:(half + 1) * 1024], po)
            else:
                nc.vector.tensor_copy(so[:, half * 1024:(half + 1) * 1024], po)

        def emit_outdma(q, so):
            nc.sync.dma_start(
                out=outT[:, q * 512:(q + 1) * 512].rearrange(
                    "(cb p) n -> p cb n", p=128),
                in_=so.rearrange("p (cb n) -> p cb n", n=512))

        for n in range(4):
            ns = slice(n * 512, (n + 1) * 512)
            accA = psB.tile([128, 512], F32, tag="acc", name=f"accA{n}")
            accB = psB.tile([128, 512], F32, tag="acc", name=f"accB{n}")
            for m in range(16):
                ms = slice(m * 128, (m + 1) * 128)
                blk = psA.tile([128, 1024], F32, tag="blk",
                               name=f"blk{n}_{m}")
                nc.tensor.matmul(blk[:, 0:512], kT[0:64, ms], qT[0:64, ns],
                                 start=True, stop=True, tile_position=(0, 0))
                nc.tensor.matmul(blk[:, 512:1024], kT[64:128, ms],
                                 qT[64:128, ns],
                                 start=True, stop=True, tile_position=(64, 0))
                P = ppool.tile([128, 1024], BF16, tag="p", name=f"p{n}_{m}")
                nc.scalar.activation(P, blk, EXP)
                # attnout scheduling: chunk 0 holds ALL its attnouts (the
                # chunk is PE-bound with the projection fills), chunk 1
                # drains the backlog at <=3/step in its PE slack; afterwards
                # a steady 3-5 step lag keeps the previous chunk's normalize
                # (reading the acc banks this chunk recycles) ahead of the
                # PE's in-order queue reaching attnout m0
                aq.append((m, P, accA, accB))
                if n == 0:
                    thresh, cap = 99, 0
                elif n == 1 and m < 6:
                    thresh, cap = 3, 3
                else:
                    thresh, cap = (4 if m in (3, 4) else 3), 2
                pops = 0
                while len(aq) > thresh and pops < cap:
                    pops += 1
                    e = aq.popleft()
                    emit_attnout(*e)
                    if e[0] == 15:
                        # chunk n-1 fully accumulated: kick its normalize
                        drain = [0, n - 1, emit_norm(n - 1, e[2], e[3]), None]
                if m >= 1 and fills[n]:
                    for task in fills[n].pop(0):
                        task()
                if drain is not None:
                    stage, dq, dstate, dso = drain
                    if stage == 0:
                        drain = [1, dq, dstate, dso]   # one-step gap for norm
                    elif stage == 1:
                        drain = [2, dq, emit_transposes(dq, dstate),
                                 opool.tile([128, 2048], F16, tag="so",
                                            name=f"so{dq}")]
                    elif stage == 2:
                        emit_outproj(dq, dstate, 0, dso)
                        drain[0] = 3
                    elif stage == 3:
                        emit_outproj(dq, dstate, 1, dso)
                        emit_outdma(dq, dso)
                        drain = None

        # ---- epilogue: drain the final chunk, pipelined per nb-pair
        # (ScalarE is idle now: it takes the h1 normalize + cb0/1 copies)
        last = None
        while aq:
            last = aq.popleft()
            emit_attnout(*last)
        # epilogue norm: h0 on DVE (split per nb-pair), h1 on the idle
        # ScalarE as per-partition-scalar multiplies -> shortest chain to
        # the first transpose
        nrm3 = npool.tile([128, 4, 128], BF16, tag="nrm", name="nrm3")
        qaccA, qaccB = last[2], last[3]
        rdA = spool.tile([128, 4], F32, tag="rd0", name="rd3_0")
        nc.vector.reciprocal(rdA, qaccA[:, 64:261:65])
        rdB = spool.tile([128, 4], F32, tag="rd1", name="rd3_1")
        nc.vector.reciprocal(rdB, qaccB[:, 64:261:65])
        avA = qaccA[:, 0:260].rearrange("p (nb c) -> p nb c", c=65)
        for half in (0, 1):
            nc.vector.tensor_tensor(
                nrm3[:, 2 * half:2 * half + 2, 0:64],
                avA[:, 2 * half:2 * half + 2, 0:64],
                rdA[:, 2 * half:2 * half + 2].to_broadcast([128, 2, 64]),
                op=MULT)
        for nb in range(4):
            nc.scalar.mul(nrm3[:, nb, 64:128],
                          qaccB[:, nb * 65: nb * 65 + 64], rdB[:, nb:nb + 1])
        tp = psA.tile([128, 512], BF16, tag="blk", name="tp3")
        at = apool.tile([128, 512], BF16, tag="attT", name="attT3")
        po = [psA.tile([128, 1024], F32, tag="blk", name=f"po3_{ph}")
              for ph in (0, 1)]
        so = opool.tile([128, 2048], F16, tag="so", name="so3")
        sor = so.rearrange("p (cb n) -> p cb n", n=512)
        for nb in range(4):
            nc.tensor.transpose(tp[:, nb * 128:(nb + 1) * 128],
                                nrm3[:, nb, :], ident)
        nc.vector.tensor_copy(at[:, 0:256], tp[:, 0:256])
        nc.vector.tensor_copy(at[:, 256:512], tp[:, 256:512])
        for nbp in (0, 1):
            for ph in (0, 1):
                for i in (0, 1):
                    cb = 2 * ph + i
                    for nb in (2 * nbp, 2 * nbp + 1):
                        nc.tensor.matmul(
                            po[ph][:, i * 512 + nb * 128:
                                   i * 512 + (nb + 1) * 128],
                            wp_sb[:, cb * 128:(cb + 1) * 128],
                            at[:, nb * 128:(nb + 1) * 128],
                            start=(nb == 0), stop=(nb == 3),
                            skip_group_check=True)
            for ph in (0, 1):
                psrc = po[ph].rearrange("p (i n) -> p i n", n=512)[
                    :, :, nbp * 256:(nbp + 1) * 256]
                pdst = sor[:, 2 * ph:2 * ph + 2, nbp * 256:(nbp + 1) * 256]
                if ph == 0:
                    nc.scalar.copy(pdst, psrc)
                else:
                    nc.vector.tensor_copy(pdst, psrc)
            nc.sync.dma_start(
                out=outT[:, 1536 + nbp * 256: 1536 + (nbp + 1) * 256
                         ].rearrange("(cb p) n -> p cb n", p=128),
                in_=sor[:, :, nbp * 256:(nbp + 1) * 256])

    nc.compile()
    return nc


def _get_program():
    global _NC
    if _NC is None:
        _NC = _build_program()
    return _NC


def make_in_maps(inputs):
    import ml_dtypes
    bf16 = ml_dtypes.bfloat16

    x = np.asarray(inputs["x"], np.float32)
    y = np.asarray(inputs["y"], np.float32)
    Wq = np.asarray(inputs["Wq"], np.float32)
    Wkv = np.asarray(inputs["Wkv"], np.float32)
    lw = np.asarray(inputs["lw"], np.float32)
    Wp = np.asarray(inputs["Wp"], np.float32)

    d = np.arange(HD)
    xr = [np.ascontiguousarray(x[b].astype(bf16)) for b in range(B)]
    yr = [np.ascontiguousarray(y[b].astype(bf16)) for b in range(B)]
    in_maps = []
    for core in range(NCORES):
        b = core // 4
        h0 = (core % 4) * 2
        ch = np.concatenate([h * HD + d for h in (h0, h0 + 1)])  # channels
        colsK = np.concatenate([h * 2 * HD + 2 * d for h in (h0, h0 + 1)])
        wq_c = Wq[:, ch] * np.float32(SCALE)
        wk_c = Wkv[:, colsK]
        wv_c = Wkv[:, colsK + 1] * (1.0 + lw[ch])[None, :]
        def pmaj(w):
            # [C, 128] -> [128, kc*128] so DMA rows are 1KB (full-rate)
            return np.ascontiguousarray(
                w.reshape(4, 128, 128).transpose(1, 0, 2).reshape(128, 512)
                .astype(bf16))
        in_maps.append({
            "xr": xr[b],
            "yr": yr[b],
            "wq": pmaj(wq_c),
            "wk": pmaj(wk_c),
            "wv": pmaj(wv_c),
            "wp": np.ascontiguousarray(Wp[ch, :].astype(bf16)),
        })
    return in_maps


def assemble_output(results, inputs):
    lb = np.asarray(inputs["lb"], np.float32)
    Wp = np.asarray(inputs["Wp"], np.float32)
    bp = np.asarray(inputs["bp"], np.float32)
    bias = (bp + lb @ Wp).astype(np.float32)
    parts = [np.asarray(results[c]["outT"], dtype=np.float32)
             for c in range(NCORES)]
    out = np.stack([parts[0] + parts[1] + parts[2] + parts[3],
                    parts[4] + parts[5] + parts[6] + parts[7]])
    out += bias[None, :, None]
    return out.astype(np.float32)


def kernel(x, y, Wq, Wkv, lw, lb, Wp, bp):
    global LAST_RUN
    from concourse.bass_utils import run_bass_kernel_spmd

    inputs = dict(x=x, y=y, Wq=Wq, Wkv=Wkv, lw=lw, lb=lb, Wp=Wp, bp=bp)
    nc = _get_program()
    in_maps = make_in_maps(inputs)
    LAST_RUN = run_bass_kernel_spmd(nc, in_maps, list(range(NCORES)))
    return assemble_output(LAST_RUN.results, inputs)


# revision 26
# speedup vs baseline: 1.0108x; 1.0108x over previous
"""Trainium2 Bass kernel for nn_CrossAttention (B=2, C=512, N=M=2048, H=8).

Sharding: batch*heads = 16 (b,h) pairs across 8 cores, 2 heads per core.
Cores 0-3 handle batch 0 (heads 0..7 in pairs), cores 4-7 batch 1.

The kernel is ScalarE-exp-bound (softmax needs 65536 exp rows/core at
0.833ns — a ~55us engine floor no other engine can take), so PE work is
restructured to fit under it:
  qT[d,n] = (Wq*SCALE).T @ x_b   (bf16)                           8192c
  kT[d,m] = Wk.T @ y_b           (bf16, f32r in SBUF)             8192c
  v2[m,d] = y_b.T-slices @ (Wv*(1+lw))  direct [m,d] layout,      8192c
            bf16, no PE transposes; ones cols give the denominator
  S^T[m,n] = kT.T-slices @ qT   (K=64 pairs tile_position-packed) 65536c
  P = exp(S^T) -> bf16          (ScalarE, 64x [128,1024] blocks)
  att[n, d|den] += P_slice.T @ v2[m]   n-major: 128 out partitions,
            65-row bf16 matmuls (half the m-major cost)           33280c
  att_nrm[n,d2] = att * recip(den)     (DVE, per-partition scalar)
  attT[d2,n] = transpose(att_nrm)      (PE, bf16 identity)         2048c
  outT_partial[c,n] = Wp_rows.T @ attT  (bf16) -> f16 partials     8192c

The depthwise conv (ksize=1) folds into Wv scaling + a host-side output
bias (bias' = bp + lb @ Wp, exact because softmax rows sum to 1).
Host sums the 4 per-batch f16 partials in f32 and adds bias'.

PSUM: psA 3x[128,1024] ring (scores/exp; also proj, v2, transposes and
outproj transients) = 6 banks; psB 2x[128,512] = 2 banks holding the
8 per-chunk attnout accumulators (4x65 cols per bank; only the first
matmul into a bank uses start=True — the bank-wide pending-zero then
zero-initializes each co-located accumulation group on first touch).

Chunk q's drain (normalize/transpose/outproj/DMA) is woven into chunk
q+1's first m-steps so the PE's in-order queue and the psA ring never
stall the score stream that feeds ScalarE.
"""

import os
import sys
import numpy as np
from contextlib import ExitStack

for _p in ("/root/.axon_site", "/root/.axon_site/_ro/trn_rl_repo",
           "/root/.axon_site/_ro/pypackages", "/opt/trn_rl_repo"):
    if os.path.isdir(_p) and _p not in sys.path:
        sys.path.append(_p)

B, C, N, M, H = 2, 512, 2048, 2048, 8
HD = C // H
SCALE = HD ** -0.5
NCORES = 8

_NC = None
LAST_RUN = None


def _build_program():
    from concourse import bacc
    import concourse.tile as tile
    import concourse.mybir as mybir
    from concourse.masks import make_identity

    F32 = mybir.dt.float32
    F32R = mybir.dt.float32r
    BF16 = mybir.dt.bfloat16
    F16 = mybir.dt.float16
    EXP = mybir.ActivationFunctionType.Exp
    MULT = mybir.AluOpType.mult

    nc = bacc.Bacc("TRN2", target_bir_lowering=False, debug=False,
                   num_devices=NCORES)

    xr = nc.dram_tensor("xr", [C, N], BF16, kind="ExternalInput").ap()
    yr = nc.dram_tensor("yr", [C, M], BF16, kind="ExternalInput").ap()
    wq_d = nc.dram_tensor("wq", [128, 512], BF16, kind="ExternalInput").ap()
    wk_d = nc.dram_tensor("wk", [128, 512], BF16, kind="ExternalInput").ap()
    wv_d = nc.dram_tensor("wv", [128, 512], BF16, kind="ExternalInput").ap()
    wp_d = nc.dram_tensor("wp", [128, C], BF16, kind="ExternalInput").ap()
    outT = nc.dram_tensor("outT", [C, N], F16, kind="ExternalOutput").ap()

    with tile.TileContext(nc) as tc, ExitStack() as ctx:
        sb = ctx.enter_context(tc.tile_pool(name="sb", bufs=1))
        ppool = ctx.enter_context(tc.tile_pool(name="ppool", bufs=20))
        npool = ctx.enter_context(tc.tile_pool(name="npool", bufs=4))
        apool = ctx.enter_context(tc.tile_pool(name="apool", bufs=2))
        spool = ctx.enter_context(tc.tile_pool(name="spool", bufs=2))
        opool = ctx.enter_context(tc.tile_pool(name="opool", bufs=2))
        psA = ctx.enter_context(tc.tile_pool(name="psA", bufs=3, space="PSUM"))
        psB = ctx.enter_context(tc.tile_pool(name="psB", bufs=2, space="PSUM"))

        # ---- constants (identity first: PE warmup gates on it) ----
        ident = sb.tile([128, 128], BF16, tag="ident")
        make_identity(nc, ident)
        # v2 group tiles: [m 128, mi 4, 130] bf16; cols 64/129 stay 1.0
        # (the softmax-denominator ones columns)
        v2g = [sb.tile([128, 4, 130], BF16, tag=f"v2g_{g}", name=f"v2g_{g}")
               for g in range(4)]

        def v2s(m, lo, hi):
            return v2g[m // 4][:, m % 4, lo:hi]
        # warm the exp table while DMAs stream
        warm = sb.tile([1, 32], F32, tag="warm")
        nc.scalar.activation(warm, ident[0:1, 0:32], EXP)
        # warm the PE clock so early projections run fast
        psw = psB.tile([128, 512], F32, tag="acc", name="psw")
        for _ in range(16):
            nc.tensor.matmul(psw[:, 0:128], ident, ident, start=True, stop=True)
        warm2 = sb.tile([128, 128], F32, tag="warm2")
        nc.vector.tensor_copy(warm2, psw[:, 0:128])
        for g in range(4):
            nc.gpsimd.memset(v2g[g], 1.0)

        wq_sb = sb.tile([128, 4, 128], BF16, tag="wq_sb")
        wk_sb = sb.tile([128, 4, 128], BF16, tag="wk_sb")
        wv_sb = sb.tile([128, 4, 128], BF16, tag="wv_sb")
        wp_sb = sb.tile([128, C], BF16, tag="wp_sb")

        y_sb = sb.tile([128, 4, M], BF16, tag="y_sb")
        x_sb = sb.tile([128, 4, N], BF16, tag="x_sb")

        def load_j(dst, src, j, kcs=None):
            js = slice(j * 512, (j + 1) * 512)
            ks = slice(0, 4) if kcs is None else slice(kcs[0], kcs[1])
            cs = slice(ks.start * 128, ks.stop * 128)
            nc.sync.dma_start(
                out=dst[:, ks, js],
                in_=src[cs, js].rearrange("(kc p) m -> p kc m", p=128))

        # DMA order = consumption order; one DMA per j-chunk.  The first
        # exp is gated by (wq,x_j0) -> qT and (wk,y_j0) -> kT; the q side
        # goes first so its ScalarE copy overlaps the k projection.
        nc.sync.dma_start(
            out=wq_sb, in_=wq_d.rearrange("p (kc m) -> p kc m", m=128))
        load_j(x_sb, xr, 0)
        nc.sync.dma_start(
            out=wk_sb, in_=wk_d.rearrange("p (kc m) -> p kc m", m=128))
        load_j(y_sb, yr, 0, kcs=(0, 3))
        load_j(y_sb, yr, 0, kcs=(3, 4))
        nc.sync.dma_start(
            out=wv_sb, in_=wv_d.rearrange("p (kc m) -> p kc m", m=128))
        load_j(y_sb, yr, 1)
        load_j(y_sb, yr, 2)
        load_j(y_sb, yr, 3)
        nc.sync.dma_start(out=wp_sb, in_=wp_d)
        load_j(x_sb, xr, 1)
        load_j(x_sb, xr, 2)
        load_j(x_sb, xr, 3)

        kT = sb.tile([128, M], F32R, tag="kT")
        qT = sb.tile([128, N], F32R, tag="qT")

        projst = {}

        def proj_half(dst, w_sb, src, j, name, half, use_act=False):
            if half == 0:
                projst[name] = psA.tile([128, 512], F32, tag="blk", name=name)
            ps = projst[name]
            for kc in (0, 1) if half == 0 else (2, 3):
                nc.tensor.matmul(ps, w_sb[:, kc, :],
                                 src[:, kc, j * 512:(j + 1) * 512],
                                 start=(kc == 0), stop=(kc == 3))
            if half == 1:
                if use_act:
                    nc.scalar.copy(dst[:, j * 512:(j + 1) * 512], ps)
                else:
                    nc.vector.tensor_copy(dst[:, j * 512:(j + 1) * 512], ps)

        def proj(dst, w_sb, src, j, name, use_act=False):
            proj_half(dst, w_sb, src, j, name, 0, use_act)
            proj_half(dst, w_sb, src, j, name, 1, use_act)

        v2st = {}

        def v2_proj2(g, half):
            # four m-blocks share one PSUM bank; only the very first matmul
            # uses start=True (bank-wide pending-zero inits the rest)
            if half == 0:
                v2st[g] = psA.tile([128, 4, 128], F32, tag="blk",
                                   name=f"psv{g}")
            ps = v2st[g]
            for mi in (0, 1) if half == 0 else (2, 3):
                m = g * 4 + mi
                for kc in range(4):
                    nc.tensor.matmul(ps[:, mi, :],
                                     y_sb[:, kc, m * 128:(m + 1) * 128],
                                     wv_sb[:, kc, :],
                                     start=(mi == 0 and kc == 0),
                                     stop=(mi == 3 and kc == 3),
                                     skip_group_check=True)
            if half == 1:
                nc.vector.tensor_copy(v2g[g][:, :, 0:64], ps[:, :, 0:64])
                nc.vector.tensor_copy(v2g[g][:, :, 65:129], ps[:, :, 64:128])

        # ---- prologue: only the j0 projections gate the first exp;
        # qT copies on ScalarE while kT's goes to DVE in parallel; the
        # kT copy is split so the m0 scores wait only on cols 0:128 ----
        proj_half(qT, wq_sb, x_sb, 0, "psq0", 0, use_act=True)
        proj_half(qT, wq_sb, x_sb, 0, "psq0", 1, use_act=True)
        proj_half(kT, wk_sb, y_sb, 0, "psk0", 0)
        for kc in (2, 3):
            nc.tensor.matmul(projst["psk0"], wk_sb[:, kc, :],
                             y_sb[:, kc, 0:512],
                             start=(kc == 0), stop=(kc == 3))
        nc.vector.tensor_copy(kT[:, 0:128], projst["psk0"][:, 0:128])
        nc.vector.tensor_copy(kT[:, 128:512], projst["psk0"][:, 128:512])

        # fill task groups woven between score blocks (chunk -> per-m lists),
        # in units of <=~450ns of PE so the score stream never starves;
        # k-proj for chunk j must be fully emitted before scores m=4j read it
        def KJ(j, half):
            return lambda: proj_half(kT, wk_sb, y_sb, j, f"psk{j}", half)

        def QJ(j, half):
            return lambda: proj_half(qT, wq_sb, x_sb, j, f"psq{j}", half)

        def VG(g, half):
            return lambda: v2_proj2(g, half)

        fills = {
            0: [[VG(0, 0), VG(0, 1)],
                [KJ(1, 0), KJ(1, 1)],
                [],
                [VG(1, 0), VG(1, 1)],
                [],
                [KJ(2, 0), KJ(2, 1)],
                [],
                [VG(2, 0), VG(2, 1)],
                [],
                [KJ(3, 0), KJ(3, 1)],
                [],
                [VG(3, 0), VG(3, 1)],
                [QJ(1, 0), QJ(1, 1)]],
            1: [[QJ(2, 0), QJ(2, 1)]],
            2: [[QJ(3, 0), QJ(3, 1)]],
            3: [],
        }

        # ---- attention main loop ----
        from collections import deque
        aq = deque()         # (m, P, accA, accB) awaiting attnout
        drain = None         # [stage, chunk, state...] of the pending drain

        def emit_attnout(pm, pP, paccA, paccB):
            # pm==0/nb==0 is the first matmul into each fresh acc bank: its
            # start=True marks the whole bank pending-zero; later groups'
            # first writes then zero-init via the per-byte pending path.
            for h, acc in ((0, paccA), (1, paccB)):
                for nb in range(4):
                    nc.tensor.matmul(
                        acc[:, nb * 65:(nb + 1) * 65],
                        pP[:, h * 512 + nb * 128: h * 512 + (nb + 1) * 128],
                        v2s(pm, h * 65, h * 65 + 65),
                        start=(pm == 0 and nb == 0),
                        stop=(pm == 15 and nb == 3),
                        skip_group_check=True)

        def emit_norm(q, qaccA, qaccB):
            # batched strided reciprocal of the 4 denominator columns per
            # bank, then ONE broadcast tensor_tensor per bank: the [128,4]
            # reciprocals broadcast (stride-0) along the 64 d-columns
            nrm = npool.tile([128, 4, 128], BF16, tag="nrm", name=f"nrm{q}")
            for h, acc in ((0, qaccA), (1, qaccB)):
                rd = spool.tile([128, 4], F32, tag=f"rd{h}", name=f"rd{q}_{h}")
                nc.vector.reciprocal(rd, acc[:, 64:261:65])
                av = acc[:, 0:260].rearrange("p (nb c) -> p nb c", c=65)
                nc.vector.tensor_tensor(
                    nrm[:, :, h * 64:(h + 1) * 64], av[:, :, 0:64],
                    rd.to_broadcast([128, 4, 64]), op=MULT)
            return nrm

        def emit_transposes(q, nrm):
            # 4 transposes share one PSUM slot; one bf16 2x copy out
            tp = psA.tile([128, 512], BF16, tag="blk", name=f"tp{q}")
            for nb in range(4):
                nc.tensor.transpose(
                    tp[:, nb * 128:(nb + 1) * 128], nrm[:, nb, :], ident)
            at = apool.tile([128, 512], BF16, tag="attT", name=f"attT{q}")
            nc.vector.tensor_copy(at, tp)
            return at

        def emit_outproj(q, at, half, so, use_act=False):
            # two output-channel blocks share one PSUM slot -> f16 halves
            po = psA.tile([128, 1024], F32, tag="blk", name=f"po{q}_{half}")
            for i in range(2):
                cb = half * 2 + i
                for nb in range(4):
                    nc.tensor.matmul(
                        po[:, i * 512 + nb * 128: i * 512 + (nb + 1) * 128],
                        wp_sb[:, cb * 128:(cb + 1) * 128],
                        at[:, nb * 128:(nb + 1) * 128],
                        start=(nb == 0), stop=(nb == 3 and i == 1),
                        skip_group_check=True)
            if use_act:
                nc.scalar.copy(so[:, half * 1024:(half + 1) * 1024], po)
            else:
                nc.vector.tensor_copy(so[:, half * 1024:(half + 1) * 1024], po)

        def emit_outdma(q, so):
            nc.sync.dma_start(
                out=outT[:, q * 512:(q + 1) * 512].rearrange(
                    "(cb p) n -> p cb n", p=128),
                in_=so.rearrange("p (cb n) -> p cb n", n=512))

        for n in range(4):
            ns = slice(n * 512, (n + 1) * 512)
            accA = psB.tile([128, 512], F32, tag="acc", name=f"accA{n}")
            accB = psB.tile([128, 512], F32, tag="acc", name=f"accB{n}")
            for m in range(16):
                ms = slice(m * 128, (m + 1) * 128)
                blk = psA.tile([128, 1024], F32, tag="blk",
                               name=f"blk{n}_{m}")
                nc.tensor.matmul(blk[:, 0:512], kT[0:64, ms], qT[0:64, ns],
                                 start=True, stop=True, tile_position=(0, 0))
                nc.tensor.matmul(blk[:, 512:1024], kT[64:128, ms],
                                 qT[64:128, ns],
                                 start=True, stop=True, tile_position=(64, 0))
                P = ppool.tile([128, 1024], BF16, tag="p", name=f"p{n}_{m}")
                nc.scalar.activation(P, blk, EXP)
                # attnout scheduling: chunk 0 holds ALL its attnouts (the
                # chunk is PE-bound with the projection fills), chunk 1
                # drains the backlog at <=3/step in its PE slack; afterwards
                # a steady 3-5 step lag keeps the previous chunk's normalize
                # (reading the acc banks this chunk recycles) ahead of the
                # PE's in-order queue reaching attnout m0
                aq.append((m, P, accA, accB))
                if n == 0:
                    thresh, cap = 99, 0
                elif n == 1 and m < 6:
                    thresh, cap = 3, 3
                else:
                    thresh, cap = (4 if m in (3, 4) else 3), 2
                pops = 0
                while len(aq) > thresh and pops < cap:
                    pops += 1
                    e = aq.popleft()
                    emit_attnout(*e)
                    if e[0] == 15:
                        # chunk n-1 fully accumulated: kick its normalize
                        drain = [0, n - 1, emit_norm(n - 1, e[2], e[3]), None]
                if m >= 1 and fills[n]:
                    for task in fills[n].pop(0):
                        task()
                if drain is not None:
                    stage, dq, dstate, dso = drain
                    if stage == 0:
                        drain = [1, dq, dstate, dso]   # one-step gap for norm
                    elif stage == 1:
                        drain = [2, dq, emit_transposes(dq, dstate),
                                 opool.tile([128, 2048], F16, tag="so",
                                            name=f"so{dq}")]
                    elif stage == 2:
                        emit_outproj(dq, dstate, 0, dso)
                        drain[0] = 3
                    elif stage == 3:
                        emit_outproj(dq, dstate, 1, dso)
                        emit_outdma(dq, dso)
                        drain = None

        # ---- epilogue: drain the final chunk, pipelined per nb-pair
        # (ScalarE is idle now: it takes the h1 normalize + cb0/1 copies)
        last = None
        while aq:
            last = aq.popleft()
            emit_attnout(*last)
        nrm3 = emit_norm(3, last[2], last[3])
        tp = psA.tile([128, 512], BF16, tag="blk", name="tp3")
        at = apool.tile([128, 512], BF16, tag="attT", name="attT3")
        po = [psA.tile([128, 1024], F32, tag="blk", name=f"po3_{ph}")
              for ph in (0, 1)]
        so = opool.tile([128, 2048], F16, tag="so", name="so3")
        sor = so.rearrange("p (cb n) -> p cb n", n=512)
        for nb in range(4):
            nc.tensor.transpose(tp[:, nb * 128:(nb + 1) * 128],
                                nrm3[:, nb, :], ident)
        nc.vector.tensor_copy(at[:, 0:256], tp[:, 0:256])
        nc.vector.tensor_copy(at[:, 256:512], tp[:, 256:512])
        for nbp in (0, 1):
            for ph in (0, 1):
                for i in (0, 1):
                    cb = 2 * ph + i
                    for nb in (2 * nbp, 2 * nbp + 1):
                        nc.tensor.matmul(
                            po[ph][:, i * 512 + nb * 128:
                                   i * 512 + (nb + 1) * 128],
                            wp_sb[:, cb * 128:(cb + 1) * 128],
                            at[:, nb * 128:(nb + 1) * 128],
                            start=(nb == 0), stop=(nb == 3),
                            skip_group_check=True)
            for ph in (0, 1):
                psrc = po[ph].rearrange("p (i n) -> p i n", n=512)[
                    :, :, nbp * 256:(nbp + 1) * 256]
                pdst = sor[:, 2 * ph:2 * ph + 2, nbp * 256:(nbp + 1) * 256]
                if ph == 0:
                    nc.scalar.copy(pdst, psrc)
                else:
                    nc.vector.tensor_copy(pdst, psrc)
            nc.sync.dma_start(
                out=outT[:, 1536 + nbp * 256: 1536 + (nbp + 1) * 256
                         ].rearrange("(cb p) n -> p cb n", p=128),
                in_=sor[:, :, nbp * 256:(nbp + 1) * 256])

    nc.compile()
    return nc


def _get_program():
    global _NC
    if _NC is None:
        _NC = _build_program()
    return _NC


def make_in_maps(inputs):
    import ml_dtypes
    bf16 = ml_dtypes.bfloat16

    x = np.asarray(inputs["x"], np.float32)
    y = np.asarray(inputs["y"], np.float32)
    Wq = np.asarray(inputs["Wq"], np.float32)
    Wkv = np.asarray(inputs["Wkv"], np.float32)
    lw = np.asarray(inputs["lw"], np.float32)
    Wp = np.asarray(inputs["Wp"], np.float32)

    d = np.arange(HD)
    xr = [np.ascontiguousarray(x[b].astype(bf16)) for b in range(B)]
    yr = [np.ascontiguousarray(y[b].astype(bf16)) for b in range(B)]
    in_maps = []
    for core in range(NCORES):
        b = core // 4
        h0 = (core % 4) * 2
        ch = np.concatenate([h * HD + d for h in (h0, h0 + 1)])  # channels
        colsK = np.concatenate([h * 2 * HD + 2 * d for h in (h0, h0 + 1)])
        wq_c = Wq[:, ch] * np.float32(SCALE)
        wk_c = Wkv[:, colsK]
        wv_c = Wkv[:, colsK + 1] * (1.0 + lw[ch])[None, :]
        def pmaj(w):
            # [C, 128] -> [128, kc*128] so DMA rows are 1KB (full-rate)
            return np.ascontiguousarray(
                w.reshape(4, 128, 128).transpose(1, 0, 2).reshape(128, 512)
                .astype(bf16))
        in_maps.append({
            "xr": xr[b],
            "yr": yr[b],
            "wq": pmaj(wq_c),
            "wk": pmaj(wk_c),
            "wv": pmaj(wv_c),
            "wp": np.ascontiguousarray(Wp[ch, :].astype(bf16)),
        })
    return in_maps


def assemble_output(results, inputs):
    lb = np.asarray(inputs["lb"], np.float32)
    Wp = np.asarray(inputs["Wp"], np.float32)
    bp = np.asarray(inputs["bp"], np.float32)
    bias = (bp + lb @ Wp).astype(np.float32)
    parts = [np.asarray(results[c]["outT"], dtype=np.float32)
             for c in range(NCORES)]
    out = np.stack([parts[0] + parts[1] + parts[2] + parts[3],
                    parts[4] + parts[5] + parts[6] + parts[7]])
    out += bias[None, :, None]
    return out.astype(np.float32)


def kernel(x, y, Wq, Wkv, lw, lb, Wp, bp):
    global LAST_RUN
    from concourse.bass_utils import run_bass_kernel_spmd

    inputs = dict(x=x, y=y, Wq=Wq, Wkv=Wkv, lw=lw, lb=lb, Wp=Wp, bp=bp)
    nc = _get_program()
    in_maps = make_in_maps(inputs)
    LAST_RUN = run_bass_kernel_spmd(nc, in_maps, list(range(NCORES)))
    return assemble_output(LAST_RUN.results, inputs)
